# revision 7
# baseline (speedup 1.0000x reference)
"""Expert-parallel DeepseekV2 MoE kernel for 8 Trainium2 NeuronCores.

Strategy:
  - Host computes the (tiny) gate routing in numpy, mirroring the reference's
    grouped top-k exactly, then gathers each expert's assigned tokens.
  - Token assignments are packed into fixed-capacity slots of 256 tokens
    (ceil(n_e/256) slots per expert). Slots are distributed round-robin over
    the 8 cores, so heavily-loaded experts are split across cores and every
    core runs an identical SPMD program with SLOTS slots.
  - Each slot streams its expert's weights once (~34.6 MB) and runs the
    SwiGLU FFN on the slot's gathered tokens.
  - The shared MLP is tensor-parallel on the intermediate dim (2816/8 = 352
    per core); every core computes a partial over all tokens.
  - Host applies combine weights (scatter-add) and sums the shared partials.

Device kernel layout notes:
  - All activations are kept transposed ([feature, token]) through the expert
    FFN so both matmuls contract on the partition dim with no transposes.
  - Matmuls use float32r (fast fp32 mode, 1 cycle/row for moving dim >= 256).
  - Weights stream through SBUF in 1.4-4 MB DMA chunks (>= 1 MiB for full HBM
    bandwidth); each weight byte is read exactly once per slot.
  - PSUM: intermediate tiles are packed two-per-bank; a bank's zero region is
    started exactly once (start=True only on the first matmul into the bank)
    and stopped once (stop=True on the last).
"""

import math

import numpy as np

import concourse.bass as bass
import concourse.tile as tile
from concourse import bacc, mybir
from concourse.bass_utils import run_bass_kernel_spmd

# Problem shapes (hardcoded per the harness contract).
T, D = 1024, 2048
E, I = 32, 1408
TOPK = 6
N_GROUP, TOPK_GROUP = 8, 3
ROUTED_SCALE = 2.5
SHARED_I = 2 * I  # 2816

NCORES = 8
ISH = SHARED_I // NCORES   # 352 shared-intermediate per core
CAP = 256                  # token capacity per slot
KT = D // 128              # 16 contraction tiles over D
IT = I // 128              # 11 intermediate tiles
TTC = CAP // 128           # 2 token tiles in the down matmul
DCH = D // 512             # 4 output chunks of 512
IS_SZ = [128, 128, ISH - 256]   # shared-intermediate tile sizes [128,128,96]

F32 = mybir.dt.float32
F32R = mybir.dt.float32r
SILU = mybir.ActivationFunctionType.Silu

_PROGRAM_CACHE = {}


def _build_program(slots):
    nc = bacc.Bacc("TRN2", target_bir_lowering=False, debug=False)

    xg = nc.dram_tensor("xg", [slots, D, CAP], F32R, kind="ExternalInput").ap()
    wg = nc.dram_tensor("wg", [slots, D, I], F32R, kind="ExternalInput").ap()
    wu = nc.dram_tensor("wu", [slots, D, I], F32R, kind="ExternalInput").ap()
    wd = nc.dram_tensor("wd", [slots, I, D], F32R, kind="ExternalInput").ap()
    xt = nc.dram_tensor("xt", [D, T], F32R, kind="ExternalInput").ap()
    wsg = nc.dram_tensor("wsg", [D, ISH], F32R, kind="ExternalInput").ap()
    wsu = nc.dram_tensor("wsu", [D, ISH], F32R, kind="ExternalInput").ap()
    wsd = nc.dram_tensor("wsd", [ISH, D], F32R, kind="ExternalInput").ap()
    ye = nc.dram_tensor("ye", [slots, CAP, D], F32, kind="ExternalOutput").ap()
    ys = nc.dram_tensor("ys", [T, D], F32, kind="ExternalOutput").ap()

    with tile.TileContext(nc) as tc, \
         tc.tile_pool(name="psum", bufs=8, space="PSUM") as psum:
        with (
            tc.tile_pool(name="shared_res", bufs=1) as shres,
            tc.tile_pool(name="shared_tmp", bufs=1) as shtmp,
            tc.tile_pool(name="ys_out", bufs=2) as yspool,
        ):
            # ---------------- shared-expert phase ----------------
            x_sb = shres.tile([128, KT, T], F32R, tag="x_sb")
            for h in range(2):
                nc.sync.dma_start(
                    out=x_sb[:, h * 8:(h + 1) * 8, :],
                    in_=xt[h * 1024:(h + 1) * 1024, :].rearrange(
                        "(a p) t -> p a t", p=128),
                )
            wsg_sb = shres.tile([128, KT, ISH], F32R, tag="wsg_sb")
            nc.sync.dma_start(out=wsg_sb[:], in_=wsg.rearrange("(a p) i -> p a i", p=128))
            wsu_sb = shres.tile([128, KT, ISH], F32R, tag="wsu_sb")
            nc.sync.dma_start(out=wsu_sb[:], in_=wsu.rearrange("(a p) i -> p a i", p=128))
            wsd_sb = shres.tile([128, 3, D], F32R, tag="wsd_sb")
            for j in range(3):
                sz = IS_SZ[j]
                nc.sync.dma_start(out=wsd_sb[:sz, j, :],
                                  in_=wsd[j * 128:j * 128 + sz, :])

            # m1: hs^T[i_s, t] = silu(wsg^T x) * (wsu^T x), tiles [<=128, 512]
            hsg_sb = shtmp.tile([128, 3, T], F32R, tag="hsg")
            hs_sb = shtmp.tile([128, 3, T], F32R, tag="hs")
            for w_sb, is_gate in ((wsg_sb, True), (wsu_sb, False)):
                ps = [psum.tile([128, 512], F32, tag="ps", name=f"ps{_i}")
                      for _i in range(6)]
                for k in range(KT):
                    for j in range(3):
                        sz = IS_SZ[j]
                        for tch in range(2):
                            nc.tensor.matmul(
                                ps[j * 2 + tch][:sz, :],
                                w_sb[:, k, j * 128:j * 128 + sz],
                                x_sb[:, k, tch * 512:(tch + 1) * 512],
                                start=(k == 0), stop=(k == KT - 1),
                            )
                for j in range(3):
                    sz = IS_SZ[j]
                    for tch in range(2):
                        tsl = slice(tch * 512, (tch + 1) * 512)
                        if is_gate:
                            nc.scalar.activation(hsg_sb[:sz, j, tsl],
                                                 ps[j * 2 + tch][:sz, :], SILU)
                        else:
                            nc.vector.tensor_mul(hs_sb[:sz, j, tsl],
                                                 ps[j * 2 + tch][:sz, :],
                                                 hsg_sb[:sz, j, tsl])

            # m2: ys[t, d] = hs^T.T @ wsd   (stationary hs^T, moving wsd)
            for tt in range(T // 128):
                ysb = yspool.tile([128, D], F32, tag="ysb")
                for dc in range(DCH):
                    py = psum.tile([128, 512], F32, tag="ps", name="pym2")
                    for j in range(3):
                        sz = IS_SZ[j]
                        nc.tensor.matmul(
                            py[:],
                            hs_sb[:sz, j, tt * 128:(tt + 1) * 128],
                            wsd_sb[:sz, j, dc * 512:(dc + 1) * 512],
                            start=(j == 0), stop=(j == 2),
                        )
                    nc.vector.tensor_copy(ysb[:, dc * 512:(dc + 1) * 512], py[:])
                nc.scalar.dma_start(out=ys[tt * 128:(tt + 1) * 128, :], in_=ysb[:])

        # ---------------- routed-expert phase ----------------
        with (
            tc.tile_pool(name="xg_pool", bufs=2) as xgpool,
            tc.tile_pool(name="wstream", bufs=4) as wpool,
            tc.tile_pool(name="wdstream", bufs=3) as wdpool,
            tc.tile_pool(name="hbufs", bufs=1) as hpool,
            tc.tile_pool(name="ye_out", bufs=2) as yepool,
        ):
            for e in range(slots):
                xg_sb = xgpool.tile([128, KT, CAP], F32R, tag="xg_sb")
                nc.sync.dma_start(out=xg_sb[:],
                                  in_=xg[e].rearrange("(a p) c -> p a c", p=128))

                hg_sb = hpool.tile([128, IT, CAP], F32R, tag="hg")
                h_sb = hpool.tile([128, IT, CAP], F32R, tag="h")
                # gate then up: h^T tiles [128(i), CAP], stationary weights.
                # PSUM banks hold two i-tiles; one start/stop per bank.
                for w_dram, is_gate in ((wg, True), (wu, False)):
                    ps = [psum.tile([128, 512], F32, tag="ps", name=f"ps{_i}")
                          for _i in range(6)]
                    for kc in range(KT // 2):
                        w_sb = wpool.tile([128, 2, I], F32R, tag="wst")
                        nc.sync.dma_start(
                            out=w_sb[:],
                            in_=w_dram[e, kc * 256:(kc + 1) * 256, :].rearrange(
                                "(a p) i -> p a i", p=128),
                        )
                        for a in range(2):
                            k = kc * 2 + a
                            for it in range(IT):
                                csl = slice((it % 2) * CAP, (it % 2) * CAP + CAP)
                                last_in_pair = (it == IT - 1) or (it % 2 == 1)
                                nc.tensor.matmul(
                                    ps[it // 2][:, csl],
                                    w_sb[:, a, it * 128:(it + 1) * 128],
                                    xg_sb[:, k, :],
                                    start=(k == 0 and it % 2 == 0),
                                    stop=(k == KT - 1 and last_in_pair),
                                )
                    for it in range(IT):
                        src = ps[it // 2][:, (it % 2) * CAP:(it % 2) * CAP + CAP]
                        if is_gate:
                            nc.scalar.activation(hg_sb[:, it, :], src, SILU)
                        else:
                            nc.vector.tensor_mul(h_sb[:, it, :], src, hg_sb[:, it, :])

                # down: y[t, d] = h^T.T @ wd  (stationary h^T, moving wd)
                ye_sb = yepool.tile([128, TTC, D], F32, tag="ye_sb")
                pys = [psum.tile([128, 512], F32, tag="ps", name=f"py{_i}")
                       for _i in range(8)]
                ichunks = [(0, 2), (2, 2), (4, 2), (6, 2), (8, 2), (10, 1)]
                for i0, cnt in ichunks:
                    wd_sb = wdpool.tile([128, 2, D], F32R, tag="wdst")
                    nc.sync.dma_start(
                        out=wd_sb[:, :cnt, :],
                        in_=wd[e, i0 * 128:(i0 + cnt) * 128, :].rearrange(
                            "(a p) d -> p a d", p=128),
                    )
                    for a in range(cnt):
                        i = i0 + a
                        for tt in range(TTC):
                            for dc in range(DCH):
                                nc.tensor.matmul(
                                    pys[tt * DCH + dc][:],
                                    h_sb[:, i, tt * 128:(tt + 1) * 128],
                                    wd_sb[:, a, dc * 512:(dc + 1) * 512],
                                    start=(i == 0), stop=(i == IT - 1),
                                )
                for tt in range(TTC):
                    for dc in range(DCH):
                        nc.vector.tensor_copy(ye_sb[:, tt, dc * 512:(dc + 1) * 512],
                                              pys[tt * DCH + dc][:])
                nc.scalar.dma_start(
                    out=ye[e].rearrange("(a p) d -> p a d", p=128), in_=ye_sb[:])

    nc.compile()
    return nc


def get_program(slots=5):
    key = ("nc", slots)
    if key not in _PROGRAM_CACHE:
        _PROGRAM_CACHE[key] = _build_program(slots)
    return _PROGRAM_CACHE[key]


def _route_numpy(x, gate_w, bias):
    """Mirror reference.py's grouped top-k routing in fp32 numpy."""
    logits = x @ gate_w                                   # [T, E]
    scores = 1.0 / (1.0 + np.exp(-logits))
    sc = scores + bias[None, :]
    g = sc.reshape(-1, N_GROUP, E // N_GROUP)
    group_scores = np.sort(g, axis=-1)[..., -2:].sum(-1)  # [T, n_group]
    gidx = np.argsort(-group_scores, axis=-1, kind="stable")[:, :TOPK_GROUP]
    gmask = np.zeros((x.shape[0], N_GROUP), np.bool_)
    np.put_along_axis(gmask, gidx, True, axis=-1)
    emask = np.repeat(gmask, E // N_GROUP, axis=-1)       # [T, E]
    masked = np.where(emask, sc, -np.inf)
    topk_idx = np.argsort(-masked, axis=-1, kind="stable")[:, :TOPK]
    w = np.take_along_axis(scores, topk_idx, axis=-1)
    w = w / (w.sum(-1, keepdims=True) + 1e-20)
    return topk_idx, w


def _plan(topk_idx, topk_w):
    """Group token assignments by expert, pack into (expert, chunk) slots,
    and deal slots round-robin across cores."""
    flat_e = topk_idx.ravel()
    flat_t = np.repeat(np.arange(T), TOPK)
    flat_w = (topk_w * ROUTED_SCALE).ravel().astype(np.float32)
    order = np.argsort(flat_e, kind="stable")
    sorted_t = flat_t[order]
    sorted_w = flat_w[order]
    counts = np.bincount(flat_e, minlength=E)
    offsets = np.concatenate([[0], np.cumsum(counts)])

    slot_list = []  # (expert, token_idx[<=CAP], weights[<=CAP])
    for e in range(E):
        toks = sorted_t[offsets[e]:offsets[e + 1]]
        ws = sorted_w[offsets[e]:offsets[e + 1]]
        for c0 in range(0, max(len(toks), 1), CAP):
            slot_list.append((e, toks[c0:c0 + CAP], ws[c0:c0 + CAP]))
    slots_per_core = max(1, math.ceil(len(slot_list) / NCORES))
    # core c gets slots c, c+8, c+16, ... (round robin)
    per_core = [[] for _ in range(NCORES)]
    for si, s in enumerate(slot_list):
        per_core[si % NCORES].append(s)
    return per_core, slots_per_core


def build_in_maps(inputs):
    """Route, pack slots, and build the per-core device input maps."""
    x = np.ascontiguousarray(np.asarray(inputs["hidden_states"], np.float32))
    gate_w = np.asarray(inputs["gate_w"], np.float32)
    bias = np.asarray(inputs["e_score_correction_bias"], np.float32)
    w_gate = np.asarray(inputs["w_gate"], np.float32)
    w_up = np.asarray(inputs["w_up"], np.float32)
    w_down = np.asarray(inputs["w_down"], np.float32)
    ws_gate = np.ascontiguousarray(np.asarray(inputs["ws_gate"], np.float32))
    ws_up = np.ascontiguousarray(np.asarray(inputs["ws_up"], np.float32))
    ws_down = np.ascontiguousarray(np.asarray(inputs["ws_down"], np.float32))

    topk_idx, topk_w = _route_numpy(x, gate_w, bias)
    per_core, slots = _plan(topk_idx, topk_w)

    x_t = np.ascontiguousarray(x.T)  # [D, T]
    in_maps = []
    for c in range(NCORES):
        xg_np = np.zeros((slots, D, CAP), np.float32)
        wg_np = np.empty((slots, D, I), np.float32)
        wu_np = np.empty((slots, D, I), np.float32)
        wd_np = np.empty((slots, I, D), np.float32)
        for j in range(slots):
            if j < len(per_core[c]):
                e, idx, _ = per_core[c][j]
            else:
                e, idx = 0, np.empty(0, np.int64)
            if len(idx):
                xg_np[j, :, :len(idx)] = x_t[:, idx]
            wg_np[j] = w_gate[e]
            wu_np[j] = w_up[e]
            wd_np[j] = w_down[e]
        in_maps.append({
            "xg": xg_np, "wg": wg_np, "wu": wu_np, "wd": wd_np,
            "xt": x_t,
            "wsg": np.ascontiguousarray(ws_gate[:, c * ISH:(c + 1) * ISH]),
            "wsu": np.ascontiguousarray(ws_up[:, c * ISH:(c + 1) * ISH]),
            "wsd": np.ascontiguousarray(ws_down[c * ISH:(c + 1) * ISH, :]),
        })
    return in_maps, per_core, slots


def kernel(**inputs):
    in_maps, per_core, slots = build_in_maps(inputs)
    nc = get_program(slots)
    res = run_bass_kernel_spmd(nc, in_maps, core_ids=list(range(NCORES)))

    routed = np.zeros((T, D), np.float32)
    shared = np.zeros((T, D), np.float32)
    for c in range(NCORES):
        for j, (e, idx, wv) in enumerate(per_core[c]):
            if not len(idx):
                continue
            y = res.results[c]["ye"][j][:len(idx)]       # [n, D]
            routed[idx] += wv[:, None] * y
        shared += res.results[c]["ys"]

    return (routed + shared).astype(np.float32)
